# revision 1
# baseline (speedup 1.0000x reference)
"""Trainium2 Bass kernel for the dynamic-kernel ECA module.

Computation per sample:
  gap  = mean(x, axis=l)                       (c,)
  h    = gelu(gap @ w1.T + b1)                 (hidden,)
  th   = tanh(h @ w2.T + b2); delta = 2*th     scalar
  k    = (5 + clip(round(delta), -3, 3)) | 1   in {3,5,7} (delta in (-2,2))
  w    = box filter of width k in 9-tap window, 1/k weights
  y    = conv1d(gap, w) along c (zero pad 4)   (c,)
  s    = sigmoid(y)
  out  = x * s[:, None]

Sharding: pure data parallel, batch 16 -> 8 cores x 2 samples.

Memory strategy (per core, x shard = 2*512*8192 f32 = 32 MB):
  x must be read for the GAP reduction before s is known, and read again
  for the final scale.  23 of the 32 [128, 2048] tiles are kept resident
  in SBUF between the two passes; only 9 are re-read.  Traffic:
  32 (read) + 9 (re-read) + 32 (write) = 73 MB/core (vs 96 naive, 64 floor).

The two samples are pipelined: sample 0's gate (MLP + conv + sigmoid,
a serial ~12 us chain of tiny ops) is computed while sample 1's loads
still stream, so the store DMAs start without a bandwidth bubble.

The data-dependent kernel size is handled without control flow: k only
takes values {3,5,7} with thresholds on th at 0.25 / -0.75, so the 9-tap
weight vector is a mask-blend of three host-precomputed candidates.
"""

import os
from contextlib import ExitStack

import numpy as np

import concourse.bacc as bacc
import concourse.mybir as mybir
import concourse.tile as tile
from concourse.tile_rust import add_dep_helper
from concourse.bass_utils import run_bass_kernel_spmd

F32 = mybir.dt.float32
ALU = mybir.AluOpType
ACTF = mybir.ActivationFunctionType
AX_X = mybir.AxisListType.X

B, C, L = 16, 512, 8192
HID = 64
N_CORES = 8
BS = B // N_CORES            # samples per core = 2
CP = C // 128                # channel chunks = 4
LCH = 2048                   # l elements per tile
LP = L // LCH                # l chunks = 4
TPS = CP * LP                # tiles per sample = 16
N_TILES = BS * TPS           # 32
N_CACHE = 23                 # tiles kept resident between the two passes


def _inst(x):
    return getattr(x, "ins", x)


def _build(b2_val):
    nc = bacc.Bacc("TRN2", target_bir_lowering=False, debug=False,
                   num_devices=N_CORES)

    x_d = nc.dram_tensor("x", [BS, C, L], F32, kind="ExternalInput").ap()
    w1t_d = nc.dram_tensor("w1t", [CP, 128, HID], F32, kind="ExternalInput").ap()
    b1_d = nc.dram_tensor("b1", [HID, 1], F32, kind="ExternalInput").ap()
    w2t_d = nc.dram_tensor("w2t", [HID, 1], F32, kind="ExternalInput").ap()
    wks_d = nc.dram_tensor("wks", [1, 27], F32, kind="ExternalInput").ap()
    id_d = nc.dram_tensor("ident", [128, 128], F32, kind="ExternalInput").ap()
    o_d = nc.dram_tensor("out", [BS, C, L], F32, kind="ExternalOutput").ap()

    with ExitStack() as ctx:
        tc = ctx.enter_context(tile.TileContext(nc))
        cache = ctx.enter_context(tc.tile_pool(name="cache", bufs=1))
        stream = ctx.enter_context(tc.tile_pool(name="stream", bufs=2))
        small = ctx.enter_context(tc.tile_pool(name="small", bufs=1))
        convp = ctx.enter_context(tc.tile_pool(name="convp", bufs=3, space="PSUM"))
        psum = ctx.enter_context(tc.tile_pool(name="psum", bufs=1, space="PSUM"))

        def tidx(s, ci, li):
            return s * TPS + ci * LP + li

        # ---- pass 1 loads + partial sums (emitted per sample) ----------
        xt = {}          # n -> cached tile
        partials = {}
        gmean = {}

        loads_emitted = []

        def load_sample(s):
            partials[s] = small.tile([128, CP, LP], F32, tag=f"partials{s}", name=f"partials{s}")
            pairs = [(ci, li) for ci in range(CP) for li in range(LP)]
            cached = [p for p in pairs if tidx(s, *p) < N_CACHE]
            streamed = [p for p in pairs if tidx(s, *p) >= N_CACHE]
            if streamed:
                # interleave so the 2 stream slots recycle throughout the
                # sample's load window instead of back-to-back at its tail
                order = []
                for i in range(max(len(cached), len(streamed))):
                    if i < len(streamed):
                        order.append(streamed[i])
                    if i < len(cached):
                        order.append(cached[i])
            else:
                order = pairs
            for ci, li in order:
                n = tidx(s, ci, li)
                if n < N_CACHE:
                    t = cache.tile([128, LCH], F32, tag=f"c{n}", name=f"c{n}")
                    xt[n] = t
                else:
                    t = stream.tile([128, LCH], F32, tag="st")
                ld = nc.sync.dma_start(
                    out=t[:],
                    in_=x_d[s, ci * 128:(ci + 1) * 128,
                            li * LCH:(li + 1) * LCH])
                loads_emitted.append(ld)
                if n < N_CACHE:
                    nc.vector.reduce_sum(out=partials[s][:, ci, li:li + 1],
                                         in_=t[:], axis=AX_X)
                else:
                    # streamed slots recycle fastest via the idle ACT
                    # engine: in-place copy with per-partition accum
                    nc.scalar.activation(
                        t[:], t[:], ACTF.Copy,
                        accum_out=partials[s][:, ci, li:li + 1])
            gmean[s] = small.tile([128, CP], F32, tag=f"gmean{s}", name=f"gmean{s}")
            nc.vector.reduce_sum(out=gmean[s][:], in_=partials[s][:],
                                 axis=AX_X)
            nc.vector.tensor_scalar_mul(gmean[s][:], gmean[s][:], 1.0 / L)

        load_sample(0)

        # ---- constants (small; needed first at sample-0's gate) --------
        w1t = small.tile([128, CP, HID], F32, tag="w1t")
        for i in range(CP):
            nc.sync.dma_start(out=w1t[:, i, :], in_=w1t_d[i])
        b1 = small.tile([HID, 1], F32, tag="b1")
        nc.sync.dma_start(out=b1[:], in_=b1_d[:])
        w2t = small.tile([HID, 1], F32, tag="w2t")
        nc.sync.dma_start(out=w2t[:], in_=w2t_d[:])
        wks = small.tile([1, 27], F32, tag="wks")
        nc.sync.dma_start(out=wks[:], in_=wks_d[:])
        ident = small.tile([128, 128], F32, tag="ident")
        nc.sync.dma_start(out=ident[:], in_=id_d[:])

        load_sample(1)

        # ---- per-sample gate: MLP, 9-tap conv, sigmoid ------------------
        def gate_sample(s):
            """Everything lives on partition 0 (or 0..127 for the psum
            transposes) so both samples use identical layouts."""
            hp = psum.tile([HID, 1], F32, tag="hp")
            for i in range(CP):
                nc.tensor.matmul(hp[:], lhsT=w1t[:, i, :],
                                 rhs=gmean[s][:, i:i + 1],
                                 start=(i == 0), stop=(i == CP - 1))
            h = small.tile([HID, 1], F32, tag="h")
            nc.scalar.activation(h[:], hp[:], ACTF.Gelu, bias=b1[:], scale=1.0)

            dp = psum.tile([1, 1], F32, tag="dp")
            nc.tensor.matmul(dp[:], lhsT=h[:], rhs=w2t[:], start=True,
                             stop=True)
            flags = small.tile([1, 4], F32, tag="flags")
            th = flags[:, 0:1]
            a = flags[:, 1:2]
            bb = flags[:, 2:3]
            u = flags[:, 3:4]
            nc.vector.tensor_scalar(out=th, in0=dp[:],
                                    scalar1=float(b2_val), scalar2=None,
                                    op0=ALU.add)
            nc.scalar.activation(th, th, ACTF.Tanh, bias=0.0, scale=1.0)

            # delta = 2*th; k = 7 iff delta >= 0.5, k = 3 iff delta < -1.5
            nc.vector.tensor_scalar(out=a, in0=th, scalar1=0.25,
                                    scalar2=None, op0=ALU.is_ge)
            nc.vector.tensor_scalar(out=bb, in0=th, scalar1=-0.75,
                                    scalar2=None, op0=ALU.is_lt)
            nc.vector.tensor_add(u, a, bb)
            nc.vector.tensor_scalar(out=u, in0=u, scalar1=-1.0,
                                    scalar2=1.0, op0=ALU.mult, op1=ALU.add)

            w18 = small.tile([1, 18], F32, tag="w18")
            wv = w18[:, 0:9]
            t9 = w18[:, 9:18]
            nc.vector.tensor_scalar(out=wv, in0=wks[:, 0:9], scalar1=bb,
                                    scalar2=None, op0=ALU.mult)
            nc.vector.tensor_scalar(out=t9, in0=wks[:, 9:18], scalar1=u,
                                    scalar2=None, op0=ALU.mult)
            nc.vector.tensor_add(wv, wv, t9)
            nc.vector.tensor_scalar(out=t9, in0=wks[:, 18:27],
                                    scalar1=a, scalar2=None, op0=ALU.mult)
            nc.vector.tensor_add(wv, wv, t9)

            # gap -> row layout [1, 520] via PE transpose (exact move)
            gpp = psum.tile([1, CP, 128], F32, tag="gpp")
            for i in range(CP):
                nc.tensor.matmul(gpp[:, i, :], lhsT=gmean[s][:, i:i + 1],
                                 rhs=ident[:], is_transpose=True,
                                 start=True, stop=True)
            gp = small.tile([1, 8 + C], F32, tag="gp")
            nc.vector.memset(gp[:], 0.0)
            nc.vector.tensor_copy(gp[:, 4:4 + C],
                                  gpp[:].rearrange("q i p -> q (i p)"))

            # 9-tap conv: muls split ACT/DVE, accumulated into y on DVE
            y = small.tile([1, C], F32, tag="y")
            for j in range(9):
                if j:
                    tcv = convp.tile([1, C], F32, tag="tc")
                else:
                    tcv = y
                if j % 2 == 0:
                    nc.scalar.mul(tcv[:], gp[:, j:j + C], wv[:, j:j + 1])
                else:
                    nc.vector.tensor_scalar(out=tcv[:], in0=gp[:, j:j + C],
                                            scalar1=wv[:, j:j + 1],
                                            scalar2=None, op0=ALU.mult)
                if j:
                    nc.vector.tensor_add(y[:], y[:], tcv[:])

            # sigmoid(y) = 0.5 + 0.5*tanh(y/2) (stays in the tanh table
            # set); computed in place in y
            sgr = y
            nc.scalar.activation(sgr[:], y[:], ACTF.Tanh, scale=0.5)
            nc.vector.tensor_scalar(out=sgr[:], in0=sgr[:], scalar1=0.5,
                                    scalar2=0.5, op0=ALU.mult, op1=ALU.add)

            # gate back to channel-major [128, ci]
            sgp = psum.tile([128, CP], F32, tag="sgp")
            for ci in range(CP):
                nc.tensor.matmul(sgp[:, ci:ci + 1],
                                 lhsT=sgr[:, ci * 128:(ci + 1) * 128],
                                 rhs=ident[0:1, 0:1], is_transpose=True,
                                 start=True, stop=True)
            sg = small.tile([128, CP], F32, tag=f"sg{s}")
            nc.vector.tensor_copy(sg[:], sgp[:])
            return sg

        # ---- pass 2: muls pre-run during the load phase; stores are
        # gated behind the last load (pure read phase, then pure-ish
        # write phase: HBM read/write interleave costs ~10% throughput).
        # Re-reads of the 10 uncached tiles recycle freed cache slots.
        sg = {}
        sg[0] = gate_sample(0)

        def mul_tile(t, s, ci, flip):
            scale_ap = sg[s][:, ci:ci + 1]
            if flip:
                nc.vector.tensor_scalar_mul(t[:], t[:], scale_ap)
            else:
                nc.scalar.mul(t[:], t[:], scale_ap)

        # in-place scale of sample-0 cached tiles (runs during s1 loads)
        for j, (ci, li) in enumerate([(c, l) for c in range(CP)
                                      for l in range(LP)]):
            mul_tile(xt[tidx(0, ci, li)], 0, ci, j % 2)

        sg[1] = gate_sample(1)
        s1_cached = [(ci, li) for ci in range(CP) for li in range(LP)
                     if tidx(1, ci, li) < N_CACHE]
        s1_streamed = [(ci, li) for ci in range(CP) for li in range(LP)
                       if tidx(1, ci, li) >= N_CACHE]
        # in-place scale of sample-1 cached tiles
        for j, (ci, li) in enumerate(s1_cached):
            mul_tile(xt[tidx(1, ci, li)], 1, ci, j % 2)

        def store_tile(t, s, ci, li):
            return nc.sync.dma_start(
                out=o_d[s, ci * 128:(ci + 1) * 128, li * LCH:(li + 1) * LCH],
                in_=t[:])

        # sample-0 stores; the first is held back until the load phase ends
        for j, (ci, li) in enumerate([(c, l) for c in range(CP)
                                      for l in range(LP)]):
            st_dma = store_tile(xt[tidx(0, ci, li)], 0, ci, li)
            if j == 0 and loads_emitted:
                gate_ld = loads_emitted[max(0, len(loads_emitted) - 3)]
                add_dep_helper(_inst(gate_ld), _inst(st_dma), sync=True,
                               reason="stores after load phase")

        # re-read the streamed tiles: the first two park in the now-idle
        # stream slots, the rest recycle cache slots freed by the stores.
        # Reads are batched before their stores to keep HBM phases pure.
        rrs = []
        for m, (ci, li) in enumerate(s1_streamed):
            if m < 2:
                t = stream.tile([128, LCH], F32, tag="st", name=f"rr{m}")
            else:
                t = cache.tile([128, LCH], F32, tag=f"c{m - 2}",
                               name=f"rr{m}")
            nc.sync.dma_start(
                out=t[:],
                in_=x_d[1, ci * 128:(ci + 1) * 128, li * LCH:(li + 1) * LCH])
            mul_tile(t, 1, ci, m % 2)
            rrs.append((t, ci, li))
        for t, ci, li in rrs:
            store_tile(t, 1, ci, li)

        # sample-1 cached stores last (their scales were ready earlier)
        for ci, li in s1_cached:
            store_tile(xt[tidx(1, ci, li)], 1, ci, li)

    nc.compile()
    return nc


_COMPILED = {}


def _get_compiled(b2_val):
    key = float(b2_val)
    if key not in _COMPILED:
        _COMPILED[key] = _build(key)
    return _COMPILED[key]


def _make_consts(w1, b1, w2, b2):
    w1 = np.asarray(w1, np.float32)
    b1 = np.asarray(b1, np.float32)
    w2 = np.asarray(w2, np.float32)
    b2 = np.asarray(b2, np.float32)
    w1t = np.ascontiguousarray(w1.T.reshape(CP, 128, HID))
    j = np.arange(9)
    cand = [(np.abs(j - 4) <= (k - 1) // 2).astype(np.float32) / np.float32(k)
            for k in (3, 5, 7)]
    wks = np.concatenate(cand).astype(np.float32)[None, :]
    return {
        "w1t": w1t,
        "b1": np.ascontiguousarray(b1.reshape(HID, 1)),
        "w2t": np.ascontiguousarray(w2.reshape(1, HID).T),
        "wks": np.ascontiguousarray(wks),
        "ident": np.eye(128, dtype=np.float32),
    }


def kernel(x, w1, b1, w2, b2):
    x = np.asarray(x, np.float32)
    assert x.shape == (B, C, L), x.shape
    nc = _get_compiled(np.float32(np.asarray(b2).reshape(-1)[0]))
    consts = _make_consts(w1, b1, w2, b2)
    in_maps = []
    for i in range(N_CORES):
        m = {"x": np.ascontiguousarray(x[i * BS:(i + 1) * BS])}
        m.update(consts)
        in_maps.append(m)
    res = run_bass_kernel_spmd(nc, in_maps, list(range(N_CORES)),
                               trace=bool(int(os.environ.get("K_TRACE", "0"))))
    out = np.concatenate([res.results[i]["out"] for i in range(N_CORES)],
                         axis=0)
    if res.exec_time_ns is not None:
        kernel.last_exec_time_ns = res.exec_time_ns
        kernel.last_mean_exec_time_ns = res.mean_exec_time_ns
    kernel.last_results = res
    return out



# revision 5
# speedup vs baseline: 1.3002x; 1.3002x over previous
"""Trainium2 Bass kernel for the dynamic-kernel ECA module.

Computation per sample:
  gap  = mean(x, axis=l)                       (c,)
  h    = gelu(gap @ w1.T + b1)                 (hidden,)
  th   = tanh(h @ w2.T + b2); delta = 2*th     scalar
  k    = (5 + clip(round(delta), -3, 3)) | 1   in {3,5,7} (delta in (-2,2))
  w    = box filter of width k in 9-tap window, 1/k weights
  y    = conv1d(gap, w) along c (zero pad 4)   (c,)
  s    = sigmoid(y)
  out  = x * s[:, None]

Sharding: pure data parallel, batch 16 -> 8 cores x 2 samples.

Memory strategy (per core, x shard = 2*512*8192 f32 = 32 MB):
  x is streamed from HBM exactly once.  While each [128, 2048] f32 tile
  is in flight through a small SBUF ring, one ACT op per tile both
  casts it into a resident fp16 cache (16 MB for all 32 tiles) and
  accumulates the per-partition sum for the GAP reduction.  After the
  per-sample gate (MLP + 9-tap conv + sigmoid) is known, each fp16
  tile is scaled back out to f32 staging and stored.  HBM traffic is
  the floor: 32 MB read + 32 MB write.  bf16 rounding of x adds
  ~2e-3 relative error (bf16 keeps f32's exponent range, so tiny x
  values stay normal; fp16 would blow up relative error below its
  6e-5 subnormal threshold).  Threshold margins of the dynamic-k gate
  are ~0.21, five orders above the gap perturbation.

  Loads and stores both ride the SP HWDGE ring (FIFO per ring), so the
  read phase and write phase stay pure; the first store is additionally
  pinned behind the tail of the load phase.  Constants load via the ACT
  HWDGE ring so the x stream starts immediately.

The data-dependent kernel size is handled without control flow: k only
takes values {3,5,7} with thresholds on th at 0.25 / -0.75, so the 9-tap
weight vector is a mask-blend of three host-precomputed candidates.
"""

import os
from contextlib import ExitStack

import numpy as np

import concourse.bacc as bacc
import concourse.mybir as mybir
import concourse.tile as tile
from concourse.tile_rust import add_dep_helper
from concourse.bass_utils import run_bass_kernel_spmd

F32 = mybir.dt.float32
F16 = mybir.dt.bfloat16
ALU = mybir.AluOpType
ACTF = mybir.ActivationFunctionType
AX_X = mybir.AxisListType.X

B, C, L = 16, 512, 8192
HID = 64
N_CORES = 8
BS = B // N_CORES            # samples per core = 2
CP = C // 128                # channel chunks = 4
LCH = 2048                   # l elements per tile
LP = L // LCH                # l chunks = 4
TPS = CP * LP                # tiles per sample = 16
N_TILES = BS * TPS           # 32
LD_BUFS = 5                  # f32 load ring
STG_BUFS = 3                 # f32 store staging ring

# OVERLAP=False: loads then stores, both on the SP ring (pure HBM phases).
# OVERLAP=True:  stores ride the ACT ring and are released as soon as each
#                sample's gate is ready, overlapping the other sample's loads.
OVERLAP = False


def _inst(x):
    return getattr(x, "ins", x)


def _build(b2_val):
    nc = bacc.Bacc("TRN2", target_bir_lowering=False, debug=False,
                   num_devices=N_CORES)

    x_d = nc.dram_tensor("x", [BS, C, L], F32, kind="ExternalInput").ap()
    w1t_d = nc.dram_tensor("w1t", [CP, 128, HID], F32, kind="ExternalInput").ap()
    b1_d = nc.dram_tensor("b1", [HID, 1], F32, kind="ExternalInput").ap()
    w2t_d = nc.dram_tensor("w2t", [HID, 1], F32, kind="ExternalInput").ap()
    wks_d = nc.dram_tensor("wks", [1, 27], F32, kind="ExternalInput").ap()
    id_d = nc.dram_tensor("ident", [128, 128], F32, kind="ExternalInput").ap()
    o_d = nc.dram_tensor("out", [BS, C, L], F32, kind="ExternalOutput").ap()

    with ExitStack() as ctx:
        tc = ctx.enter_context(tile.TileContext(nc))
        cache = ctx.enter_context(tc.tile_pool(name="cache", bufs=1))
        ring = ctx.enter_context(tc.tile_pool(name="ring", bufs=LD_BUFS))
        small = ctx.enter_context(tc.tile_pool(name="small", bufs=1))
        convp = ctx.enter_context(tc.tile_pool(name="convp", bufs=3, space="PSUM"))
        psum = ctx.enter_context(tc.tile_pool(name="psum", bufs=1, space="PSUM"))

        def tidx(s, ci, li):
            return s * TPS + ci * LP + li

        # ---- constants ride the ACT HWDGE ring (SP ring is all-x) ------
        w1t = small.tile([128, CP, HID], F32, tag="w1t")
        for i in range(CP):
            nc.scalar.dma_start(out=w1t[:, i, :], in_=w1t_d[i])
        b1 = small.tile([HID, 1], F32, tag="b1")
        nc.scalar.dma_start(out=b1[:], in_=b1_d[:])
        w2t = small.tile([HID, 1], F32, tag="w2t")
        nc.scalar.dma_start(out=w2t[:], in_=w2t_d[:])
        wks = small.tile([1, 27], F32, tag="wks")
        nc.scalar.dma_start(out=wks[:], in_=wks_d[:])
        ident = small.tile([128, 128], F32, tag="ident")
        nc.scalar.dma_start(out=ident[:], in_=id_d[:])

        # ---- pass 1: stream x once; cast to resident fp16 + reduce -----
        xt = {}          # n -> fp16 cache tile
        partials = {}
        gmean = {}
        loads = []

        def load_sample(s):
            partials[s] = small.tile([128, CP, LP], F32, tag=f"partials{s}",
                                     name=f"partials{s}")
            for ci in range(CP):
                for li in range(LP):
                    n = tidx(s, ci, li)
                    t = ring.tile([128, LCH], F32, tag="ld", name=f"ld{n}")
                    ld = nc.sync.dma_start(
                        out=t[:],
                        in_=x_d[s, ci * 128:(ci + 1) * 128,
                                li * LCH:(li + 1) * LCH])
                    loads.append(ld)
                    xc = cache.tile([128, LCH], F16, tag=f"c{n}", name=f"c{n}")
                    xt[n] = xc
                    # one ACT op: fp16 cast + per-partition sum
                    nc.scalar.activation(
                        xc[:], t[:], ACTF.Copy,
                        accum_out=partials[s][:, ci, li:li + 1])
            gmean[s] = small.tile([128, CP], F32, tag=f"gmean{s}",
                                  name=f"gmean{s}")
            nc.vector.reduce_sum(out=gmean[s][:], in_=partials[s][:],
                                 axis=AX_X)
            nc.vector.tensor_scalar_mul(gmean[s][:], gmean[s][:], 1.0 / L)

        # ---- per-sample gate: MLP, 9-tap conv, sigmoid ------------------
        def gate_sample(s):
            """Everything lives on partition 0 (or 0..127 for the psum
            transposes) so both samples use identical layouts."""
            hp = psum.tile([HID, 1], F32, tag="hp")
            for i in range(CP):
                nc.tensor.matmul(hp[:], lhsT=w1t[:, i, :],
                                 rhs=gmean[s][:, i:i + 1],
                                 start=(i == 0), stop=(i == CP - 1))
            h = small.tile([HID, 1], F32, tag="h")
            nc.scalar.activation(h[:], hp[:], ACTF.Gelu, bias=b1[:], scale=1.0)

            dp = psum.tile([1, 1], F32, tag="dp")
            nc.tensor.matmul(dp[:], lhsT=h[:], rhs=w2t[:], start=True,
                             stop=True)
            flags = small.tile([1, 4], F32, tag="flags")
            th = flags[:, 0:1]
            a = flags[:, 1:2]
            bb = flags[:, 2:3]
            u = flags[:, 3:4]
            nc.vector.tensor_scalar(out=th, in0=dp[:],
                                    scalar1=float(b2_val), scalar2=None,
                                    op0=ALU.add)
            nc.scalar.activation(th, th, ACTF.Tanh, bias=0.0, scale=1.0)

            # delta = 2*th; k = 7 iff delta >= 0.5, k = 3 iff delta < -1.5
            nc.vector.tensor_scalar(out=a, in0=th, scalar1=0.25,
                                    scalar2=None, op0=ALU.is_ge)
            nc.vector.tensor_scalar(out=bb, in0=th, scalar1=-0.75,
                                    scalar2=None, op0=ALU.is_lt)
            nc.vector.tensor_add(u, a, bb)
            nc.vector.tensor_scalar(out=u, in0=u, scalar1=-1.0,
                                    scalar2=1.0, op0=ALU.mult, op1=ALU.add)

            w18 = small.tile([1, 18], F32, tag="w18")
            wv = w18[:, 0:9]
            t9 = w18[:, 9:18]
            nc.vector.tensor_scalar(out=wv, in0=wks[:, 0:9], scalar1=bb,
                                    scalar2=None, op0=ALU.mult)
            nc.vector.tensor_scalar(out=t9, in0=wks[:, 9:18], scalar1=u,
                                    scalar2=None, op0=ALU.mult)
            nc.vector.tensor_add(wv, wv, t9)
            nc.vector.tensor_scalar(out=t9, in0=wks[:, 18:27],
                                    scalar1=a, scalar2=None, op0=ALU.mult)
            nc.vector.tensor_add(wv, wv, t9)

            # gap -> row layout [1, 520] via PE transpose (exact move)
            gpp = psum.tile([1, CP, 128], F32, tag="gpp")
            for i in range(CP):
                nc.tensor.matmul(gpp[:, i, :], lhsT=gmean[s][:, i:i + 1],
                                 rhs=ident[:], is_transpose=True,
                                 start=True, stop=True)
            gp = small.tile([1, 8 + C], F32, tag="gp")
            nc.vector.memset(gp[:], 0.0)
            nc.vector.tensor_copy(gp[:, 4:4 + C],
                                  gpp[:].rearrange("q i p -> q (i p)"))

            # 9-tap conv: muls split ACT/DVE, accumulated into y on DVE
            y = small.tile([1, C], F32, tag="y")
            for j in range(9):
                if j:
                    tcv = convp.tile([1, C], F32, tag="tc")
                else:
                    tcv = y
                if j % 2 == 0:
                    nc.scalar.mul(tcv[:], gp[:, j:j + C], wv[:, j:j + 1])
                else:
                    nc.vector.tensor_scalar(out=tcv[:], in0=gp[:, j:j + C],
                                            scalar1=wv[:, j:j + 1],
                                            scalar2=None, op0=ALU.mult)
                if j:
                    nc.vector.tensor_add(y[:], y[:], tcv[:])

            # sigmoid(y) = 0.5 + 0.5*tanh(y/2) (stays in the tanh table
            # set); computed in place in y
            sgr = y
            nc.scalar.activation(sgr[:], y[:], ACTF.Tanh, scale=0.5)
            nc.vector.tensor_scalar(out=sgr[:], in0=sgr[:], scalar1=0.5,
                                    scalar2=0.5, op0=ALU.mult, op1=ALU.add)

            # gate back to channel-major [128, ci]
            sgp = psum.tile([128, CP], F32, tag="sgp")
            for ci in range(CP):
                nc.tensor.matmul(sgp[:, ci:ci + 1],
                                 lhsT=sgr[:, ci * 128:(ci + 1) * 128],
                                 rhs=ident[0:1, 0:1], is_transpose=True,
                                 start=True, stop=True)
            sg = small.tile([128, CP], F32, tag=f"sg{s}")
            nc.vector.tensor_copy(sg[:], sgp[:])
            return sg

        sg = {}
        load_sample(0)
        sg[0] = gate_sample(0)
        load_sample(1)
        sg[1] = gate_sample(1)

        # ---- pass 2: scale fp16 cache into f32 staging, store ----------
        def store_sample(s, gate_first):
            first = True
            for ci in range(CP):
                for li in range(LP):
                    n = tidx(s, ci, li)
                    g = ring.tile([128, LCH], F32, tag="stg", name=f"stg{n}",
                                  bufs=STG_BUFS)
                    scale_ap = sg[s][:, ci:ci + 1]
                    if n % 2:
                        nc.vector.tensor_scalar(out=g[:], in0=xt[n][:],
                                                scalar1=scale_ap,
                                                scalar2=None, op0=ALU.mult)
                    else:
                        nc.scalar.mul(g[:], xt[n][:], scale_ap)
                    eng = nc.scalar if OVERLAP else nc.sync
                    st = eng.dma_start(
                        out=o_d[s, ci * 128:(ci + 1) * 128,
                                li * LCH:(li + 1) * LCH],
                        in_=g[:])
                    if first and gate_first:
                        add_dep_helper(_inst(loads[-3]), _inst(st), sync=True,
                                       reason="stores after load phase")
                    first = False

        store_sample(0, gate_first=not OVERLAP)
        store_sample(1, gate_first=False)

    nc.compile()
    return nc


_COMPILED = {}


def _get_compiled(b2_val):
    key = float(b2_val)
    if key not in _COMPILED:
        _COMPILED[key] = _build(key)
    return _COMPILED[key]


def _make_consts(w1, b1, w2, b2):
    w1 = np.asarray(w1, np.float32)
    b1 = np.asarray(b1, np.float32)
    w2 = np.asarray(w2, np.float32)
    b2 = np.asarray(b2, np.float32)
    w1t = np.ascontiguousarray(w1.T.reshape(CP, 128, HID))
    j = np.arange(9)
    cand = [(np.abs(j - 4) <= (k - 1) // 2).astype(np.float32) / np.float32(k)
            for k in (3, 5, 7)]
    wks = np.concatenate(cand).astype(np.float32)[None, :]
    return {
        "w1t": w1t,
        "b1": np.ascontiguousarray(b1.reshape(HID, 1)),
        "w2t": np.ascontiguousarray(w2.reshape(1, HID).T),
        "wks": np.ascontiguousarray(wks),
        "ident": np.eye(128, dtype=np.float32),
    }


def kernel(x, w1, b1, w2, b2):
    x = np.asarray(x, np.float32)
    assert x.shape == (B, C, L), x.shape
    nc = _get_compiled(np.float32(np.asarray(b2).reshape(-1)[0]))
    consts = _make_consts(w1, b1, w2, b2)
    in_maps = []
    for i in range(N_CORES):
        m = {"x": np.ascontiguousarray(x[i * BS:(i + 1) * BS])}
        m.update(consts)
        in_maps.append(m)
    res = run_bass_kernel_spmd(nc, in_maps, list(range(N_CORES)),
                               trace=bool(int(os.environ.get("K_TRACE", "0"))))
    out = np.concatenate([res.results[i]["out"] for i in range(N_CORES)],
                         axis=0)
    if res.exec_time_ns is not None:
        kernel.last_exec_time_ns = res.exec_time_ns
        kernel.last_mean_exec_time_ns = res.mean_exec_time_ns
    kernel.last_results = res
    return out
